# revision 1
# baseline (speedup 1.0000x reference)
"""DiffAttention (differential multi-head attention) Bass kernel, single TRN2
NeuronCore, minimal-I/O edition.

Why single-core: execution goes through an axon-proxied PJRT device where each
dispatch pays a ~73 ms fixed floor plus ~90 us per MB of kernel I/O (inputs and
outputs, summed over cores). On-device compute is orders of magnitude below
the floor (~2.7 ms for the whole problem on one core), so the optimal shape is
ONE core with the smallest possible I/O footprint:
  - no replication of x (an 8-core head-parallel split ships x 8x),
  - fp16 for all large tensors (x, wq, wk, wv, wout, output),
  - small operands packed into two tensors (trig tables, subln/lambda).
Total staged I/O is ~58 MB vs ~464 MB for the 8-core fp32 variant.

Kernel structure: 8 sequential "phases", each identical to one core's slice of
the old head-parallel design (4 q/k heads = 2 differential heads per phase):
  phase1: q/k/v projections (fp16 matmuls, fp32 PSUM) + RoPE -> qTr/kTr fp16
  phase2: scores (fp16, *sqrt(128) folded into the exp), unnormalized softmax
          (exp -> bf16), attention with a ones-column in v producing rowsums,
          lambda-combine + RMSNorm epilogue -> per-head attnT tiles (fp16)
Then one dense out-projection pass contracts all 16 heads' attnT against wout.

Layout notes (carried over from the 8-core version):
  - x is passed transposed (xT [E, T]) so q/k projections come out as
    qT/kT [head_dim, T] (matmul operands for scores) without on-chip
    transposes.
  - wq/wk rows are permuted host-side so each 64-dim head comes out
    de-interleaved ([32 real | 32 imag] RoPE halves). Scores are invariant to
    a common q/k head-dim permutation.
  - Softmax is unnormalized on-chip: e = exp(s); each head's value matrix
    carries an extra ones-column so the attention matmul produces both e@v and
    rowsum(e); normalization, the diff-attn lambda combine and RMSNorm happen
    on the small [t,128] attn tiles.
  - rsqrt for RMSNorm is exp(-0.5*ln(x)) so ScalarE only ever needs the
    natural_log_exp_and_others activation table (no table thrash with the
    softmax exps).
"""

import numpy as np

import concourse.bacc as bacc
import concourse.mybir as mybir
from concourse.tile import TileContext
from concourse.masks import make_identity

# Force every ScalarE activation onto the one table set that contains all the
# functions this kernel uses (Exp, Ln, Copy): natural_log_exp_and_others.
# The default chooser takes the first covering set per function, which
# alternates exp_and_others / natural_log and costs ~2.7us per switch.
_orig_get_tables = bacc.get_activation_tables


def _single_set_tables(arch):
    tabs = _orig_get_tables(arch)
    keep = "natural_log_exp_and_others"
    if keep in tabs:
        tabs = {k: (v if k == keep else set()) for k, v in tabs.items()}
    return tabs


bacc.get_activation_tables = _single_set_tables

E = 2048            # embed dim
T = 2048            # sequence length
HALF = 64           # q/k head dim
NH2 = 32            # q/k heads
H = 16              # differential heads
PHASES = 8
HPP = H // PHASES           # diff heads per phase (2)
QPP = 2 * HPP               # q/k heads per phase (4)
DPP = QPP * HALF            # q/k feature rows per phase (256)
FPP = HPP * 2 * HALF        # v/attn feature cols per phase (256)
DEPTH = 12
LAMBDA_INIT = 0.8 - 0.6 * float(np.exp(-0.3 * DEPTH))
SQRT_HD = float((2 * HALF) ** 0.5)   # scores are multiplied by sqrt(128)
EPS = 1e-5

F32 = mybir.dt.float32
F16 = mybir.dt.float16
BF16 = mybir.dt.bfloat16
AF = mybir.ActivationFunctionType

TSUP = 512          # wide tile (moving free dim of most matmuls)
NT = T // TSUP      # 4
NE = E // 128       # 16 contraction chunks over embed dim
NS = T // 128       # 16 s (key position) chunks
TW = 1024           # scores/exp super-tile width (2 PSUM banks)
NTW = T // TW       # 2
VW = 2 * HALF + 2   # 130: v columns per head + ones column + pad (8B psum align)

# wcat column-block offsets: [wq | wk | wv | woutT]
WQ_OFF, WK_OFF, WV_OFF, WO_OFF = 0, E, 2 * E, 3 * E


def build_nc():
    nc = bacc.Bacc("TRN2", target_bir_lowering=False, debug=False)

    xT = nc.dram_tensor("xT", [E, T], F16, kind="ExternalInput").ap()
    wcat = nc.dram_tensor("wcat", [E, 4 * E], F16, kind="ExternalInput").ap()
    trig = nc.dram_tensor("trig", [256, T], F16, kind="ExternalInput").ap()
    smalls = nc.dram_tensor("smalls", [17 * 128, 1], F32, kind="ExternalInput").ap()
    outT = nc.dram_tensor("outT", [E, T], F16, kind="ExternalOutput").ap()

    with TileContext(nc) as tc:
        with (
            tc.tile_pool(name="consts", bufs=1) as consts,
            tc.tile_pool(name="persist", bufs=1) as persist,
        ):
            # rotating prefetch pool: phase p's first weight piece (contraction
            # chunks 0-3) is DMA'd during phase p-1's attention section, so the
            # projection matmuls at each phase start never wait on HBM. Phase
            # 0's piece is emitted before everything else so the big startup
            # loads hit the DMA queue first; none of the consts below are
            # needed until well into phase 0.
            wpre_cm = tc.tile_pool(name="wpre", bufs=2)
            wpre = wpre_cm.__enter__()
            pre = _emit_w_prefetch(nc, wpre, wcat, 0)

            # ---- constants ----
            ident = consts.tile([128, 128], F32, tag="ident")
            make_identity(nc, ident)

            # trig stays fp16 in SBUF too (DVE widens on read); halving these
            # two [128, T] tiles is what lets epool run at 44 bufs
            cos_t = consts.tile([128, T], F16, tag="cos")
            sin_t = consts.tile([128, T], F16, tag="sin")
            nc.sync.dma_start(out=cos_t, in_=trig[0:128, :])
            nc.sync.dma_start(out=sin_t, in_=trig[128:256, :])

            sub_t = []
            for h in range(H):
                st = consts.tile([128, 1], F32, tag=f"sub{h}", name=f"sub{h}")
                nc.sync.dma_start(out=st, in_=smalls[h * 128:(h + 1) * 128, :])
                sub_t.append(st)
            lam_bc = consts.tile([128, 1], F32, tag="lam_bc")
            nc.sync.dma_start(out=lam_bc, in_=smalls[16 * 128:17 * 128, :])
            eps_t = consts.tile([128, 1], F32, tag="eps_t")
            nc.vector.memset(eps_t, float(EPS))

            # ---- persistent activations ----
            qTr = [persist.tile([128, T], F16, tag=f"qTr{i}", name=f"qTr{i}") for i in range(2)]
            kTr = [persist.tile([128, T], F16, tag=f"kTr{i}", name=f"kTr{i}") for i in range(2)]
            v_ext = [persist.tile([128, HPP * VW], BF16, tag=f"vext{i}", name=f"vext{i}") for i in range(NS)]
            attnT = [persist.tile([128, T], F16, tag=f"attnT{h}", name=f"attnT{h}") for h in range(H)]

            for ph in range(PHASES):
                pre = emit_phase(nc, tc, ph, xT, wcat, cos_t, sin_t, sub_t,
                                 lam_bc, eps_t, ident, qTr, kTr, v_ext,
                                 attnT, wpre, pre)
            wpre_cm.__exit__(None, None, None)

            # ================= final: out projection =================
            with (
                tc.tile_pool(name="wopool", bufs=2) as wopool,
                tc.tile_pool(name="obuf", bufs=4) as obuf,
                tc.tile_pool(name="ops", bufs=2, space="PSUM") as ops,
            ):
                for eb in range(NE):
                    wo = wopool.tile([128, H * 128], F16, tag="wo", name=f"wo{eb}")
                    nc.sync.dma_start(
                        out=wo.rearrange("p (h e) -> p h e", h=H),
                        in_=wcat[:, WO_OFF + eb * 128:WO_OFF + (eb + 1) * 128]
                        .rearrange("(h p) e -> p h e", p=128))
                    for jj in range(NT):
                        js = slice(jj * TSUP, (jj + 1) * TSUP)
                        O = ops.tile([128, TSUP], F32, tag="out", name=f"o{eb}_{jj}")
                        for h in range(H):
                            nc.tensor.matmul(O, lhsT=wo[:, h * 128:(h + 1) * 128],
                                             rhs=attnT[h][:, js],
                                             start=(h == 0), stop=(h == H - 1))
                        Ob = obuf.tile([128, TSUP], F16, tag="ob", name=f"ob{eb}_{jj}")
                        if jj % 2 == 0:
                            nc.scalar.activation(out=Ob, in_=O, func=AF.Copy)
                        else:
                            nc.vector.tensor_copy(out=Ob, in_=O)
                        nc.sync.dma_start(out=outT[eb * 128:(eb + 1) * 128, js], in_=Ob)

    nc.finalize()
    return nc


def _emit_w_prefetch(nc, wpre, wcat, ph):
    """Allocate + DMA phase ph's first weight piece (contraction chunks 0-3)
    into the rotating prefetch pool."""
    wsrc = {"wq": WQ_OFF, "wk": WK_OFF, "wv": WV_OFF}
    pre = {}
    for nm in ("wq", "wk", "wv"):
        t = wpre.tile([128, 4 * DPP], F16, tag=nm, name=f"{nm}_pre{ph}")
        src = wcat[:, wsrc[nm] + ph * DPP:wsrc[nm] + (ph + 1) * DPP]
        nc.sync.dma_start(
            out=t.rearrange("p (i c) -> p i c", i=4),
            in_=src.rearrange("(i p) c -> p i c", p=128)[:, 0:4])
        pre[nm] = t
    return pre


def emit_phase(nc, tc, ph, xT, wcat, cos_t, sin_t, sub_t, lam_bc, eps_t, ident,
               qTr, kTr, v_ext, attnT, wpre, pre):
    """One phase = projections + RoPE + scores/softmax/attention for q/k heads
    [4*ph, 4*ph+4) i.e. differential heads {2*ph, 2*ph+1}. `pre` holds this
    phase's prefetched weight chunks 0-3; returns the next phase's."""

    # ================= phase 1: q/k/v projections + RoPE =================
    with (
        tc.tile_pool(name="wpool", bufs=1) as wpool,
        tc.tile_pool(name="p1x", bufs=12) as p1x,
        tc.tile_pool(name="p1tmp", bufs=3) as p1tmp,
        tc.tile_pool(name="p1ps", bufs=1, space="PSUM") as p1ps,
    ):
        # weight slices beyond the prefetched piece, chunk-major in the free
        # dim: wbig cols [(i-4)*W : (i-3)*W] = contraction chunk i.
        wbig = {}
        wsrc = {"wq": WQ_OFF, "wk": WK_OFF, "wv": WV_OFF}
        for nm in ("wq", "wk", "wv"):
            wbig[nm] = wpool.tile([128, (NE - 4) * DPP], F16, tag=nm, name=f"{nm}_{ph}")

        def emit_w_piece(piece):
            isl = slice(piece * 4, (piece + 1) * 4)
            osl = slice(piece * 4 - 4, piece * 4)
            for nm in ("wq", "wk", "wv"):
                src = wcat[:, wsrc[nm] + ph * DPP:wsrc[nm] + (ph + 1) * DPP]
                nc.sync.dma_start(
                    out=wbig[nm].rearrange("p (i c) -> p i c", i=NE - 4)[:, osl],
                    in_=src.rearrange("(i p) c -> p i c", p=128)[:, isl])

        def w_slice(nm, i, lo, hi):
            if i < 4:
                return pre[nm][:, i * DPP + lo:i * DPP + hi]
            return wbig[nm][:, (i - 4) * DPP + lo:(i - 4) * DPP + hi]

        def load_x(j, tag):
            # x chunk-pair tiles: cols [c*TSUP:(c+1)*TSUP] = e-chunk 2*ip+c
            js = slice(j * TSUP, (j + 1) * TSUP)
            xts = []
            for ip in range(NE // 2):
                xt = p1x.tile([128, 2 * TSUP], F16, tag=tag, name=f"{tag}{ph}_{j}_{ip}")
                nc.sync.dma_start(
                    out=xt.rearrange("p (c t) -> p c t", c=2),
                    in_=xT[2 * ip * 128:(2 * ip + 2) * 128, js].rearrange(
                        "(c p) t -> p c t", p=128))
                xts.append(xt)
            return xts

        def rope(src, dst, j):
            # out = P*cos + swap32(P)*signed_sin
            js = slice(j * TSUP, (j + 1) * TSUP)
            ps = p1tmp.tile([128, TSUP], F32, tag="ps", name=f"ps{ph}_{j}")
            nc.scalar.activation(out=ps, in_=src, func=AF.Copy)
            swp = p1tmp.tile([128, TSUP], F32, tag="swp", name=f"swp{ph}_{j}")
            for gsel in range(4):
                o = gsel * 32
                so = o ^ 32
                nc.gpsimd.tensor_copy(out=swp[o:o + 32, :], in_=ps[so:so + 32, :])
            t1 = p1tmp.tile([128, TSUP], F32, tag="t1", name=f"t1_{ph}_{j}")
            nc.vector.tensor_mul(out=t1, in0=ps, in1=cos_t[:, js])
            t2 = p1tmp.tile([128, TSUP], F32, tag="t2", name=f"t2_{ph}_{j}")
            nc.vector.tensor_mul(out=t2, in0=swp, in1=sin_t[:, js])
            nc.vector.tensor_add(out=dst[:, js], in0=t1, in1=t2)

        # --- pass K: k projection + RoPE for all of T first, so the score
        # matmuls (which read kTr across the full sequence) never wait on the
        # last RoPE chain: the Q/V pass below hides it. ---
        for j in range(NT):
            Pk = [p1ps.tile([128, TSUP], F32, tag=f"pk{d}", name=f"pk{d}_{ph}_{j}") for d in range(2)]
            xts = load_x(j, "xk")
            if j == 0:
                # queued after the first weight piece + x tiles so the
                # first projection matmuls start as early as possible
                for piece in range(1, 4):
                    emit_w_piece(piece)
            for i in range(NE):
                xsl = xts[i // 2][:, (i % 2) * TSUP:(i % 2 + 1) * TSUP]
                for d in range(2):
                    nc.tensor.matmul(Pk[d], lhsT=w_slice("wk", i, d * 128, (d + 1) * 128),
                                     rhs=xsl, start=(i == 0), stop=(i == NE - 1))
            rope(Pk[0], kTr[0], j)
            rope(Pk[1], kTr[1], j)

        # --- pass QV ---
        for j in range(NT):
            Pq = [p1ps.tile([128, TSUP], F32, tag=f"pq{d}", name=f"pq{d}_{ph}_{j}") for d in range(2)]
            Pv = [p1ps.tile([128, FPP], F32, tag=f"pv{sb}", name=f"pv{sb}_{ph}_{j}") for sb in range(4)]
            xts = load_x(j, "x")
            for i in range(NE):
                xsl = xts[i // 2][:, (i % 2) * TSUP:(i % 2 + 1) * TSUP]
                for d in range(2):
                    nc.tensor.matmul(Pq[d], lhsT=w_slice("wq", i, d * 128, (d + 1) * 128),
                                     rhs=xsl, start=(i == 0), stop=(i == NE - 1))
                for sb in range(4):
                    nc.tensor.matmul(Pv[sb], lhsT=xsl[:, sb * 128:(sb + 1) * 128],
                                     rhs=w_slice("wv", i, 0, FPP),
                                     start=(i == 0), stop=(i == NE - 1))
            rope(Pq[0], qTr[0], j)
            rope(Pq[1], qTr[1], j)
            # --- v psum drain into bf16 v_ext (+ ones/pad columns) ---
            for sb in range(4):
                vt = v_ext[4 * j + sb]
                for h in range(HPP):
                    nc.scalar.activation(out=vt[:, h * VW:h * VW + 128],
                                         in_=Pv[sb][:, h * 128:(h + 1) * 128],
                                         func=AF.Copy)
                    # col 128: ones (rowsum trick); col 129: pad
                    nc.gpsimd.memset(vt[:, h * VW + 128:h * VW + 129], 1.0)
                    nc.gpsimd.memset(vt[:, h * VW + 129:h * VW + 130], 0.0)

    # ============ phase 2: scores, softmax, attention ============
    # prefetch the next phase's first weight piece; its DMA runs while the
    # engines chew on this phase's attention
    pre_next = (_emit_w_prefetch(nc, wpre, wcat, ph + 1)
                if ph + 1 < PHASES else None)
    with (
        tc.tile_pool(name="epool", bufs=44) as epool,
        tc.tile_pool(name="epi", bufs=4) as epi,
        tc.tile_pool(name="p2ps", bufs=2, space="PSUM") as p2ps,
        tc.tile_pool(name="p2pa", bufs=3, space="PSUM") as p2pa,
        tc.tile_pool(name="p2pt", bufs=1, space="PSUM") as p2pt,
    ):
        def emit_attn_unit(j2, h, et, tb):
            # both diff-attn component heads accumulate into one
            # psum bank: [e0@{v|1} | e1@{v|1}]
            gh = HPP * ph + h         # global differential head index
            A = p2pa.tile([128, 2 * VW], F32, tag="attn", name=f"a{ph}_{j2}_{h}_{tb}")
            for m in range(2):
                for i in range(NS):
                    nc.tensor.matmul(
                        A[:, m * VW:(m + 1) * VW],
                        lhsT=et[(m, i)][:, tb * 128:(tb + 1) * 128],
                        rhs=v_ext[i][:, h * VW:(h + 1) * VW],
                        start=(i == 0), stop=(i == NS - 1))
            # epilogue: normalize, diff, RMSNorm
            sfx = f"{ph}_{j2}{h}{tb}"
            rho0 = epi.tile([128, 1], F32, tag="rho0", name=f"r0_{sfx}")
            nc.vector.reciprocal(out=rho0, in_=A[:, 128:129])
            rho1 = epi.tile([128, 1], F32, tag="rho1", name=f"r1_{sfx}")
            nc.vector.reciprocal(out=rho1, in_=A[:, VW + 128:VW + 129])
            nc.vector.tensor_mul(out=rho1, in0=rho1, in1=lam_bc)
            d0 = epi.tile([128, 128], F32, tag="d0", name=f"d0_{sfx}")
            nc.vector.tensor_scalar_mul(out=d0, in0=A[:, 0:128], scalar1=rho0)
            d1 = epi.tile([128, 128], F32, tag="d1", name=f"d1_{sfx}")
            nc.vector.tensor_scalar_mul(out=d1, in0=A[:, VW:VW + 128], scalar1=rho1)
            nc.vector.tensor_sub(out=d0, in0=d0, in1=d1)
            sq = epi.tile([128, 128], F32, tag="sq", name=f"sq_{sfx}")
            nc.vector.tensor_mul(out=sq, in0=d0, in1=d0)
            ss = epi.tile([128, 1], F32, tag="ss", name=f"ss_{sfx}")
            nc.vector.reduce_sum(out=ss, in_=sq, axis=mybir.AxisListType.X)
            # rsqrt(mean+eps) = exp(-0.5*ln(sum/128 + eps))
            nc.scalar.activation(out=ss, in_=ss, func=AF.Ln,
                                 bias=eps_t, scale=1.0 / 128)
            nc.scalar.activation(out=ss, in_=ss, func=AF.Exp, scale=-0.5)
            af = epi.tile([128, 128], F32, tag="af", name=f"af_{sfx}")
            nc.vector.tensor_scalar_mul(out=af, in0=d0, scalar1=ss)
            Tp = p2pt.tile([128, 128], F32, tag="tp", name=f"tp_{sfx}")
            nc.tensor.transpose(Tp, af, ident)
            tcol = (j2 * TW // 128 + tb) * 128
            # transposed tile rows are attn features -> fold the
            # per-feature subln weight in here (per-partition scalar)
            nc.vector.tensor_scalar_mul(
                out=attnT[gh][:, tcol:tcol + 128], in0=Tp, scalar1=sub_t[gh])

        # Software pipeline: the attention units of head (j2,h) are
        # emitted interleaved into the front half of the NEXT head's
        # score/exp stream, so the PE's in-order stream alternates
        # ScalarE-paced score matmuls with dense attention matmuls.
        # pending is flushed before the phase ends (the next phase
        # overwrites qTr/kTr/v_ext).
        pending = None
        for j2 in range(NTW):
            for h in range(HPP):
                et = {}
                idx = 0
                for m in range(2):
                    g = 2 * h + m
                    gt, go = g // 2, 64 * (g % 2)
                    for i in range(NS):
                        S = p2ps.tile([128, TW], F32, tag="score",
                                      name=f"s{ph}_{j2}_{h}_{m}_{i}")
                        for hf in range(2):
                            ts = slice(j2 * TW + hf * TSUP, j2 * TW + (hf + 1) * TSUP)
                            nc.tensor.matmul(
                                S[:, hf * TSUP:(hf + 1) * TSUP],
                                lhsT=kTr[gt][go:go + 64, i * 128:(i + 1) * 128],
                                rhs=qTr[gt][go:go + 64, ts], start=True, stop=True)
                        e = epool.tile([128, TW], BF16, tag="e",
                                       name=f"e{ph}_{j2}_{h}_{m}_{i}")
                        nc.scalar.activation(out=e, in_=S, func=AF.Exp, scale=SQRT_HD)
                        et[(m, i)] = e
                        if pending is not None and idx < 16 and idx % 2 == 1:
                            pj2, ph_, pet = pending
                            emit_attn_unit(pj2, ph_, pet, idx // 2)
                        idx += 1
                pending = (j2, h, et)
        pj2, ph_, pet = pending
        for tb in range(TW // 128):
            emit_attn_unit(pj2, ph_, pet, tb)
    return pre_next


_NC_CACHE = []


def _get_nc():
    if not _NC_CACHE:
        _NC_CACHE.append(build_nc())
    return _NC_CACHE[0]


class _CachedRunner:
    """Builds the jitted single-device executable once and reuses it."""

    def __init__(self, nc):
        import jax
        from concourse import bass2jax, mybir as _mb

        bass2jax.install_neuronx_cc_hook()
        self.nc = nc
        partition_name = nc.partition_id_tensor.name if nc.partition_id_tensor else None
        in_names, out_names, out_avals = [], [], []
        for alloc in nc.m.functions[0].allocations:
            if not isinstance(alloc, _mb.MemoryLocationSet):
                continue
            name = alloc.memorylocations[0].name
            if alloc.kind == "ExternalInput":
                if name != partition_name:
                    in_names.append(name)
            elif alloc.kind == "ExternalOutput":
                out_names.append(name)
                out_avals.append(jax.core.ShapedArray(
                    tuple(alloc.tensor_shape), _mb.dt.np(alloc.dtype)))
        self.in_names, self.out_names, self.out_avals = in_names, out_names, out_avals
        all_names = in_names + out_names
        if partition_name is not None:
            all_names = all_names + [partition_name]

        def _body(*args):
            operands = list(args)
            if partition_name is not None:
                operands.append(bass2jax.partition_id_tensor())
            outs = bass2jax._bass_exec_p.bind(
                *operands,
                out_avals=tuple(out_avals),
                in_names=tuple(all_names),
                out_names=tuple(out_names),
                lowering_input_output_aliases=(),
                sim_require_finite=True,
                sim_require_nnan=True,
                nc=nc,
            )
            return tuple(outs)

        self._device = jax.devices()[0]
        self._fn = jax.jit(_body, device=self._device, keep_unused=True)
        self._jax = jax

    def concat_inputs(self, in_map):
        args = [np.asarray(in_map[n]) for n in self.in_names]
        for av in self.out_avals:
            args.append(np.zeros(av.shape, av.dtype))
        return args

    def device_put(self, args):
        return [self._jax.device_put(a, self._device) for a in args]

    def run(self, args):
        outs = self._fn(*args)
        return [np.asarray(o) for o in outs]

    def __call__(self, in_map):
        outs = self.run(self.concat_inputs(in_map))
        return {n: outs[i] for i, n in enumerate(self.out_names)}


_RUNNER_CACHE = []


def _get_runner():
    if not _RUNNER_CACHE:
        _RUNNER_CACHE.append(_CachedRunner(_get_nc()))
    return _RUNNER_CACHE[0]


def _prep_inputs(x, wq, wk, wv, wout, lambda_q1, lambda_q2, lambda_k1, lambda_k2,
                 subln_weight):
    x = np.asarray(x, np.float32).reshape(T, E)
    xT = np.ascontiguousarray(x.T.astype(np.float16))

    inv = 1.0 / (10000.0 ** (np.arange(0, HALF, 2)[: HALF // 2].astype(np.float64) / HALF))
    ang = np.outer(np.arange(T), inv)          # [T, 32]
    cos32 = np.cos(ang).T.astype(np.float32)   # [32, T]
    sin32 = np.sin(ang).T.astype(np.float32)
    trig = np.empty((256, T), np.float16)
    trig[0:128] = np.tile(cos32, (4, 1))
    trig[128:256] = np.concatenate([-sin32, sin32, -sin32, sin32], axis=0)

    lam1 = float(np.exp(np.sum(np.asarray(lambda_q1, np.float64)
                               * np.asarray(lambda_k1, np.float64))))
    lam2 = float(np.exp(np.sum(np.asarray(lambda_q2, np.float64)
                               * np.asarray(lambda_k2, np.float64))))
    lam = lam1 - lam2 + LAMBDA_INIT
    smalls = np.empty((17 * 128, 1), np.float32)
    smalls[0:16 * 128, 0] = np.tile(np.asarray(subln_weight, np.float32), H)
    smalls[16 * 128:, 0] = lam

    evens = np.arange(0, HALF, 2)
    odds = np.arange(1, HALF, 2)
    deint = np.concatenate([evens, odds])
    perm = np.concatenate([g * HALF + deint for g in range(NH2)])

    wq = np.asarray(wq, np.float32)
    wk = np.asarray(wk, np.float32)
    wv = np.asarray(wv, np.float32)
    wout = np.asarray(wout, np.float32)

    wcat = np.empty((E, 4 * E), np.float16)
    wcat[:, WQ_OFF:WQ_OFF + E] = wq[perm, :].T
    wcat[:, WK_OFF:WK_OFF + E] = wk[perm, :].T
    wcat[:, WV_OFF:WV_OFF + E] = wv.T
    wcat[:, WO_OFF:WO_OFF + E] = wout.T   # rows = attn features, cols = E
    return dict(xT=xT, wcat=wcat, trig=trig, smalls=smalls)


def kernel(**inputs):
    runner = _get_runner()
    in_map = _prep_inputs(**inputs)
    out = runner(in_map)["outT"]
    return np.ascontiguousarray(out.astype(np.float32).T).reshape(1, T, E)

